# revision 23
# baseline (speedup 1.0000x reference)
"""Trainium2 Bass kernel for nn_ActionNetwork (dense_mlp, 8-core data parallel).

Layout strategy: feature-on-partition, batch-on-free-dim.
  - Host transposes x [B,80] -> xT [80,B] (feature rows reordered to
    [queue(64), vehicle(8), mini(8)]), shards batch across 8 cores, bf16.
  - The potential network (2 tiny linears) is linear in x, so it is folded
    into one [80,64] stationary producing diff*DP directly from xT.
  - All broadcasts (over j), reductions (row/col sums) and the diag scatter
    are tiny PE matmuls with precomputed 0/1-weighted bf16 stationaries.
  - Elementwise chain is split DVE/ACT/Pool over [128, 512] tiles packing
    2 batch-groups (64 (i,j)-pairs each).
  - Narrow 8-wide per-node quantities pack 2 groups contiguously (rows 0:32
    resp 0:16) in one PSUM bank; group-dependent zero-padded stationaries
    let each per-group xT matmul land on its own row range while later
    2-group matmuls accumulate the whole range at once.
Output written as outT [128, Bc] (rows 0:64 action_ij, 64:128 price_ij),
host transposes back and permutes columns into the reference interleaved
[action_i | price_i] * 8 ordering.
"""

import os
import sys

import numpy as np

sys.path.insert(0, "/opt/trn_rl_repo")

N = 8
B = 262144
NCORES = 8
BC = B // NCORES  # 32768 per core
F = 512           # batch elements per group (one PSUM bank of f32)
STB = F * 4       # batch per supertile = 2048 (2 pairs x 2 groups)
NST = BC // STB   # 16 supertiles per core

LAST_RESULT = None  # test harness reads exec_time_ns from here

OLD_OF_NEW = np.concatenate([np.arange(16, 80), np.arange(0, 8),
                             np.arange(8, 16)])  # xT feature row order


def build_consts(W0, b0, W1, b1, DP, QP, DepF, ArrF, mf, IntF, PF):
    """Build all constant matrices (float64, logical layout).

    xT feature order is [queue(64), vehicle(8), mini(8)]:
      rows 0:64 queue_ij, 64:72 vehicle_i, 72:80 mini_i.
    """
    W0 = np.asarray(W0, np.float64)
    W1 = np.asarray(W1, np.float64)
    b0 = np.asarray(b0, np.float64)
    b1 = np.asarray(b1, np.float64)
    DP = np.asarray(DP, np.float64)
    QP = np.asarray(QP, np.float64)
    DepF = np.asarray(DepF, np.float64)
    ArrF = np.asarray(ArrF, np.float64)
    mf = np.asarray(mf, np.float64)
    IntF = np.asarray(IntF, np.float64)
    PF = np.asarray(PF, np.float64)

    W0full = np.zeros((80, 8))  # ORIGINAL x feature order first
    for i in range(8):
        W0full[i, i] = W0[i, 0]
        W0full[8 + i, i] = W0[i, 1]
        for j in range(8):
            W0full[16 + i * 8 + j, i] += W0[i, 2 + j]        # queue[i, j]
            W0full[16 + j * 8 + i, i] += W0[i, 2 + 8 + j]    # queue[j, i]
    Wp = W0full @ W1.T            # [80, 8]
    bp = b0 @ W1.T + b1           # [8]

    C = {}
    # Wd[f, ij] = (Wp[f,i] - Wp[f,j]) * DP[i,j], in new row order
    Wd = (Wp[:, :, None] - Wp[:, None, :]).reshape(80, 64) * DP.reshape(64)[None, :]
    C["Wd"] = Wd[OLD_OF_NEW]
    C["bias_d"] = ((bp[:, None] - bp[None, :]) * DP).reshape(64)

    S_qqp = np.zeros((80, 64))   # new row order directly: queue at rows 0:64
    for ij in range(64):
        S_qqp[ij, ij] = QP.reshape(64)[ij]
    C["S_qqp"] = S_qqp

    # smalls A stationary per group h: [80, 32], content at cols 16h:16h+16
    for h in range(2):
        S = np.zeros((80, 32))
        for i in range(8):
            S[64 + i, 16 * h + i] = 1.0             # V_i
            for j in range(8):
                S[i * 8 + j, 16 * h + i] = -QP[i, j]
                S[i * 8 + j, 16 * h + 8 + i] = QP[i, j]
        C[f"S_sm{h}"] = S
    # smalls B stationary per group h: [80, 16], content at cols 8h:8h+8
    for h in range(2):
        S = np.zeros((80, 16))
        for j in range(8):
            S[64 + j, 8 * h + j] = 1.0              # V_j
            S[72 + j, 8 * h + j] = mf[j, 0]         # mini*mf
        C[f"S_va{h}"] = S

    # V broadcast over j (from xT): SVB[64+i, i*8+j] = 1
    SVB = np.zeros((80, 64))
    for i in range(8):
        for j in range(8):
            SVB[64 + i, i * 8 + j] = 1.0
    C["SVB"] = SVB

    C["I128"] = np.eye(128)

    # rowsum(g0) into smalls A, 2-group: [128, 32]
    R2 = np.zeros((128, 32))
    for h in range(2):
        for i in range(8):
            for j in range(8):
                R2[h * 64 + i * 8 + j, 16 * h + i] = -1.0
                R2[h * 64 + i * 8 + j, 16 * h + 8 + i] = 1.0
    C["R2"] = R2

    # diag scatter of remain, 2-group: [32, 128]
    DG = np.zeros((32, 128))
    for h in range(2):
        for i in range(8):
            DG[16 * h + i, 64 * h + i * 8 + i] = 1.0
    C["DIAG"] = DG

    # tot broadcast (remain+rsg over j), 2-group: [32, 128]
    TB = np.zeros((32, 128))
    for h in range(2):
        for k in range(16):
            i = k % 8
            for j in range(8):
                TB[16 * h + k, 64 * h + i * 8 + j] = 1.0
    C["TOTB"] = TB

    # fv accumulation from raw (-DepF rowsum + ArrF colsum), 2-group [128, 16]
    CD = np.zeros((128, 16))
    RI = np.zeros((128, 16))
    for h in range(2):
        for i in range(8):
            for j in range(8):
                CD[h * 64 + i * 8 + j, 8 * h + i] += -DepF[i, j]
                CD[h * 64 + i * 8 + j, 8 * h + j] += ArrF[i, j]
                RI[h * 64 + i * 8 + j, 8 * h + i] += -IntF[i, j]
    C["CARD"] = CD
    C["RINT"] = RI

    # no_remain broadcast with PF weight, 2-group [16, 128]
    BP = np.zeros((16, 128))
    for h in range(2):
        for i in range(8):
            for j in range(8):
                BP[8 * h + i, 64 * h + i * 8 + j] = PF[i, j]
    C["B1PF"] = BP

    C["negPF"] = -PF.reshape(64)
    return C


def numpy_model(x, C):
    """Pure-numpy emulation of the device dataflow (algebra validation).
    x: [B, 80] ORIGINAL order. Returns [B,128]: 0:64 action_ij, 64:128 price."""
    x = np.asarray(x, np.float32)[:, OLD_OF_NEW].astype(np.float64)
    Bn = x.shape[0]
    diff = x @ C["Wd"] + C["bias_d"]
    g0 = np.maximum(diff, 0.0)
    t2 = x @ C["S_qqp"]
    smA = x @ C["S_sm0"][:, 0:16] + g0 @ C["R2"][0:64, 0:16]
    sm = np.maximum(smA, 0.0)
    remain, rsg = sm[:, 0:8], sm[:, 8:16]
    gradient = g0 + t2
    gradient[:, 0::9] += remain
    tot_b = np.concatenate([remain, rsg], 1) @ C["TOTB"][0:16, 0:64]
    action = gradient / tot_b
    vb = x @ C["SVB"]
    raw = action * vb
    fg = gradient - raw
    fq = np.maximum(x[:, 0:64] - raw, 0.0)
    smB = x @ C["S_va0"][:, 0:8] + raw @ C["CARD"][0:64, 0:8] \
        + fg @ C["RINT"][0:64, 0:8]
    nr = np.maximum(smB * (1.0 / 7.0), 0.0)
    nrb = nr @ C["B1PF"][0:8, 0:64]
    i1 = fg - fq
    u1 = i1 * C["negPF"] + 1.0
    u2 = u1 - nrb
    price = np.maximum(np.minimum(u2, 1.0), 0.6)
    out = np.empty((Bn, 128), np.float32)
    out[:, 0:64] = action
    out[:, 64:128] = price
    return out


def ref_col_perm():
    """perm such that final[:, c] = mine[:, perm[c]] matches reference layout."""
    perm = np.empty(128, np.int64)
    for i in range(8):
        for j in range(8):
            perm[i * 16 + j] = i * 8 + j
            perm[i * 16 + 8 + j] = 64 + i * 8 + j
    return perm


# device constant blob: pack all bf16 matrices as [128, ncols] column blocks
_BLOB_SPECS = [  # (name, rows, cols)
    ("Wd", 80, 64), ("S_qqp", 80, 64), ("S_sm0", 80, 32), ("S_sm1", 80, 32),
    ("S_va0", 80, 16), ("S_va1", 80, 16), ("SVB", 80, 64), ("I128", 128, 128),
    ("R2", 128, 32), ("DIAG", 32, 128), ("TOTB", 32, 128),
    ("CARD", 128, 16), ("RINT", 128, 16), ("B1PF", 16, 128),
]
_BLOB_OFF = {}
_off = 0
for _n, _r, _c in _BLOB_SPECS:
    _BLOB_OFF[_n] = (_off, _r, _c)
    _off += _c
BLOB_COLS = _off


def pack_blob(C):
    import ml_dtypes

    blob = np.zeros((128, BLOB_COLS), np.float32)
    for n, (o, r, c) in _BLOB_OFF.items():
        blob[0:r, o:o + c] = C[n]
    vec = np.zeros((128, 2), np.float32)
    vec[0:64, 0] = C["bias_d"]
    vec[64:128, 0] = C["bias_d"]
    vec[0:64, 1] = C["negPF"]
    vec[64:128, 1] = C["negPF"]
    return np.ascontiguousarray(blob).astype(ml_dtypes.bfloat16), vec


def _build_nc():
    import concourse.bacc as bacc
    import concourse.tile as tile
    from concourse import mybir

    f32 = mybir.dt.float32
    bf16 = mybir.dt.bfloat16
    RELU = mybir.ActivationFunctionType.Relu
    ALU = mybir.AluOpType

    nc = bacc.Bacc()
    xt_d = nc.declare_dram_parameter("xt", [80, BC], bf16, isOutput=False)
    cst_d = nc.declare_dram_parameter("cst", [128, BLOB_COLS], bf16, isOutput=False)
    cstv_d = nc.declare_dram_parameter("cstv", [128, 2], f32, isOutput=False)
    out_d = nc.declare_dram_parameter("out", [128, BC], f32, isOutput=True)

    with tile.TileContext(nc) as tc:
        with (
            tc.tile_pool(name="const", bufs=1) as cpool,
            tc.tile_pool(name="io", bufs=4) as iopool,
            tc.tile_pool(name="work", bufs=5) as wpool,
            tc.tile_pool(name="ps1", bufs=1, space="PSUM") as pp1,
            tc.tile_pool(name="ps2", bufs=3, space="PSUM") as pp2,
        ):
            cst = cpool.tile([128, BLOB_COLS], bf16)
            cstv = cpool.tile([128, 2], f32)
            nc.sync.dma_start(out=cst[:, :], in_=cst_d[:, :])
            nc.sync.dma_start(out=cstv[:, :], in_=cstv_d[:, :])

            def cs(name):
                o, r, c = _BLOB_OFF[name]
                return cst[0:r, o:o + c]

            biasd_a = cstv[0:128, 0:1]
            negpf_a = cstv[0:128, 1:2]

            for st in range(NST):
                b0c = st * STB
                xt = iopool.tile([80, STB], bf16, tag="xt")
                nc.sync.dma_start(out=xt[:, :], in_=xt_d[:, b0c:b0c + STB])

                for pr in range(2):
                    diffb = pp1.tile([128, F], f32, tag="diff")
                    gradb = pp2.tile([128, F], f32, tag="grad")
                    g0t = wpool.tile([128, F], bf16, tag="g0")
                    sm_sb = wpool.tile([32, F], bf16, tag="sm_sb")
                    smb_sb = wpool.tile([16, F], bf16, tag="smb_sb")

                    for h in range(2):
                        g = pr * 2 + h
                        xg = xt[:, g * F:(g + 1) * F]
                        po = h * 64
                        nc.tensor.matmul(out=diffb[po:po + 64, :], lhsT=cs("Wd"),
                                         rhs=xg, start=True, stop=True)
                        nc.tensor.matmul(out=gradb[po:po + 64, :], lhsT=cs("S_qqp"),
                                         rhs=xg, start=True, stop=False)
                    # g0 = relu(diff + bias_d) on ACT
                    nc.scalar.activation(out=g0t[:, :], in_=diffb[:, :], func=RELU,
                                         bias=biasd_a, scale=1.0)
                    # smalls A bank: filled late (shortens PSUM tenure)
                    sma = pp1.tile([128, F], f32, tag="sma")
                    for h in range(2):
                        g = pr * 2 + h
                        nc.tensor.matmul(out=sma[0:32, :], lhsT=cs(f"S_sm{h}"),
                                         rhs=xt[:, g * F:(g + 1) * F],
                                         start=(h == 0), stop=False)
                    # gradient += g0 ; smalls A += rowsum(g0)
                    nc.tensor.matmul(out=gradb[:, :], lhsT=cs("I128"), rhs=g0t[:, :],
                                     start=False, stop=False)
                    nc.tensor.matmul(out=sma[0:32, :], lhsT=cs("R2"), rhs=g0t[:, :],
                                     start=False, stop=True)
                    # smalls A: {remain, rsg} = relu(sma) on ACT
                    nc.scalar.activation(out=sm_sb[:, :], in_=sma[0:32, :], func=RELU)

                    totb = pp1.tile([128, F], f32, tag="bcast")
                    actt = wpool.tile([128, F], f32, tag="act")
                    rawt = wpool.tile([128, F], bf16, tag="raw")
                    fgt = wpool.tile([128, F], bf16, tag="fg")
                    fq0 = wpool.tile([128, F], bf16, tag="fq0")
                    fqt = wpool.tile([128, F], bf16, tag="fq")
                    rtb = wpool.tile([128, F], f32, tag="rtb")
                    q2 = wpool.tile([128, F], bf16, tag="q2")
                    # gradient += diag(remain); tot_b — single 2-group MMs
                    nc.tensor.matmul(out=gradb[:, :], lhsT=cs("DIAG"),
                                     rhs=sm_sb[0:32, :], start=False, stop=True)
                    nc.tensor.matmul(out=totb[:, :], lhsT=cs("TOTB"),
                                     rhs=sm_sb[0:32, :], start=True, stop=True)
                    nc.vector.reciprocal_approx_fast(out=rtb[:, :], in_=totb[:, :])
                    # V broadcast bank: filled just before use
                    vbb = pp1.tile([128, F], f32, tag="vb")
                    for h in range(2):
                        g = pr * 2 + h
                        nc.tensor.matmul(out=vbb[h * 64:h * 64 + 64, :],
                                         lhsT=cs("SVB"),
                                         rhs=xt[:, g * F:(g + 1) * F],
                                         start=True, stop=True)
                    nc.vector.tensor_mul(out=actt[:, :], in0=gradb[:, :], in1=rtb[:, :])
                    nc.vector.tensor_mul(out=rawt[:, :], in0=actt[:, :], in1=vbb[:, :])
                    nc.vector.tensor_sub(out=fgt[:, :], in0=gradb[:, :], in1=rawt[:, :])
                    for h in range(2):
                        g = pr * 2 + h
                        nc.gpsimd.tensor_copy(out=q2[h * 64:h * 64 + 64, :],
                                              in_=xt[0:64, g * F:(g + 1) * F])
                    # fq = relu(Q - raw): sub on DVE, relu on ACT
                    nc.vector.tensor_sub(out=fq0[:, :], in0=q2[:, :], in1=rawt[:, :])
                    nc.scalar.activation(out=fqt[:, :], in_=fq0[:, :], func=RELU)
                    # smalls B bank: filled late, right before its accumulators
                    smb = pp1.tile([128, F], f32, tag="smb")
                    for h in range(2):
                        g = pr * 2 + h
                        nc.tensor.matmul(out=smb[0:16, :], lhsT=cs(f"S_va{h}"),
                                         rhs=xt[:, g * F:(g + 1) * F],
                                         start=(h == 0), stop=False)
                    nc.tensor.matmul(out=smb[0:16, :], lhsT=cs("CARD"),
                                     rhs=rawt[:, :], start=False, stop=False)
                    nc.tensor.matmul(out=smb[0:16, :], lhsT=cs("RINT"),
                                     rhs=fgt[:, :], start=False, stop=True)
                    # smalls B: no_remain = relu(fv_minus_int / 7) on ACT
                    nc.scalar.activation(out=smb_sb[:, :], in_=smb[0:16, :],
                                         func=RELU, scale=1.0 / 7.0)

                    nrb = pp1.tile([128, F], f32, tag="bcast")
                    i1t = wpool.tile([128, F], bf16, tag="i1")
                    prt = wpool.tile([128, F], f32, tag="prc")
                    nc.tensor.matmul(out=nrb[:, :], lhsT=cs("B1PF"),
                                     rhs=smb_sb[0:16, :], start=True, stop=True)
                    nc.vector.tensor_sub(out=i1t[:, :], in0=fgt[:, :], in1=fqt[:, :])
                    # u1 = 1 - i1*PF ; u2 = u1 - nrb ; price = clamp(u2, .6, 1)
                    nc.vector.tensor_scalar(out=i1t[:, :], in0=i1t[:, :],
                                            scalar1=negpf_a, scalar2=1.0,
                                            op0=ALU.mult, op1=ALU.add)
                    nc.vector.tensor_sub(out=prt[:, :], in0=i1t[:, :], in1=nrb[:, :])
                    nc.vector.tensor_scalar(out=prt[:, :], in0=prt[:, :],
                                            scalar1=1.0, scalar2=0.6,
                                            op0=ALU.min, op1=ALU.max)
                    for h in range(2):
                        g = pr * 2 + h
                        po = h * 64
                        bg = b0c + g * F
                        nc.sync.dma_start(out=out_d[0:64, bg:bg + F],
                                          in_=actt[po:po + 64, :])
                        nc.sync.dma_start(out=out_d[64:128, bg:bg + F],
                                          in_=prt[po:po + 64, :])
    nc.finalize()
    return nc


_CACHE = {}


def kernel(**inputs):
    global LAST_RESULT
    x = np.ascontiguousarray(np.asarray(inputs["x"], np.float32))
    C = build_consts(
        inputs["W0"], inputs["b0"], inputs["W1"], inputs["b1"],
        inputs["distribute_param"], inputs["queue_param"],
        inputs["departure_factor"], inputs["arrival_factor"],
        inputs["mini_factor"], inputs["intention_factor"],
        inputs["price_factor"],
    )
    blob, vec = pack_blob(C)

    if "nc" not in _CACHE:
        _CACHE["nc"] = _build_nc()
    nc = _CACHE["nc"]

    from concourse.bass_utils import run_bass_kernel_spmd

    import ml_dtypes

    bf = ml_dtypes.bfloat16
    in_maps = []
    for c in range(NCORES):
        shard = np.ascontiguousarray(
            x[c * BC:(c + 1) * BC][:, OLD_OF_NEW].T).astype(bf)
        in_maps.append({"xt": shard, "cst": blob, "cstv": vec})

    trace = bool(int(os.environ.get("KBENCH_TRACE", "0")))
    if trace:
        try:
            import ntff_shim

            ntff_shim.install()
        except Exception as e:  # profiling is best-effort
            print(f"ntff shim install failed: {e}")
    res = run_bass_kernel_spmd(nc, in_maps, core_ids=list(range(NCORES)),
                               trace=trace)
    LAST_RESULT = res

    perm = ref_col_perm()
    out = np.empty((B, 128), np.float32)
    for c in range(NCORES):
        mine = res.results[c]["out"].T  # [BC, 128] in my column order
        out[c * BC:(c + 1) * BC, :] = mine[:, perm]
    return out


# revision 25
# speedup vs baseline: 1.0403x; 1.0403x over previous
"""Trainium2 Bass kernel for nn_ActionNetwork (dense_mlp, 8-core data parallel).

Layout strategy: feature-on-partition, batch-on-free-dim.
  - Host transposes x [B,80] -> xT [80,B] (feature rows reordered to
    [queue(64), vehicle(8), mini(8)]), shards batch across 8 cores, bf16.
  - The potential network (2 tiny linears) is linear in x, so it is folded
    into one [80,64] stationary producing diff*DP directly from xT.
  - All broadcasts (over j), reductions (row/col sums) and the diag scatter
    are tiny PE matmuls with precomputed 0/1-weighted bf16 stationaries.
  - Elementwise chain is split DVE/ACT/Pool over [128, 512] tiles packing
    2 batch-groups (64 (i,j)-pairs each).
  - Narrow 8-wide per-node quantities pack 2 groups contiguously (rows 0:32
    resp 0:16) in one PSUM bank; group-dependent zero-padded stationaries
    let each per-group xT matmul land on its own row range while later
    2-group matmuls accumulate the whole range at once.
Output written as outT [128, Bc] (rows 0:64 action_ij, 64:128 price_ij),
host transposes back and permutes columns into the reference interleaved
[action_i | price_i] * 8 ordering.
"""

import os
import sys

import numpy as np

sys.path.insert(0, "/opt/trn_rl_repo")

N = 8
B = 262144
NCORES = 8
BC = B // NCORES  # 32768 per core
F = 512           # batch elements per group (one PSUM bank of f32)
STB = F * 4       # batch per supertile = 2048 (2 pairs x 2 groups)
NST = BC // STB   # 16 supertiles per core

LAST_RESULT = None  # test harness reads exec_time_ns from here

OLD_OF_NEW = np.concatenate([np.arange(16, 80), np.arange(0, 8),
                             np.arange(8, 16)])  # xT feature row order


def build_consts(W0, b0, W1, b1, DP, QP, DepF, ArrF, mf, IntF, PF):
    """Build all constant matrices (float64, logical layout).

    xT feature order is [queue(64), vehicle(8), mini(8)]:
      rows 0:64 queue_ij, 64:72 vehicle_i, 72:80 mini_i.
    """
    W0 = np.asarray(W0, np.float64)
    W1 = np.asarray(W1, np.float64)
    b0 = np.asarray(b0, np.float64)
    b1 = np.asarray(b1, np.float64)
    DP = np.asarray(DP, np.float64)
    QP = np.asarray(QP, np.float64)
    DepF = np.asarray(DepF, np.float64)
    ArrF = np.asarray(ArrF, np.float64)
    mf = np.asarray(mf, np.float64)
    IntF = np.asarray(IntF, np.float64)
    PF = np.asarray(PF, np.float64)

    W0full = np.zeros((80, 8))  # ORIGINAL x feature order first
    for i in range(8):
        W0full[i, i] = W0[i, 0]
        W0full[8 + i, i] = W0[i, 1]
        for j in range(8):
            W0full[16 + i * 8 + j, i] += W0[i, 2 + j]        # queue[i, j]
            W0full[16 + j * 8 + i, i] += W0[i, 2 + 8 + j]    # queue[j, i]
    Wp = W0full @ W1.T            # [80, 8]
    bp = b0 @ W1.T + b1           # [8]

    C = {}
    # Wd[f, ij] = (Wp[f,i] - Wp[f,j]) * DP[i,j], in new row order
    Wd = (Wp[:, :, None] - Wp[:, None, :]).reshape(80, 64) * DP.reshape(64)[None, :]
    C["Wd"] = Wd[OLD_OF_NEW]
    C["bias_d"] = ((bp[:, None] - bp[None, :]) * DP).reshape(64)

    S_qqp = np.zeros((80, 64))   # new row order directly: queue at rows 0:64
    for ij in range(64):
        S_qqp[ij, ij] = QP.reshape(64)[ij]
    C["S_qqp"] = S_qqp

    # smalls A stationary per group h: [80, 32], content at cols 16h:16h+16
    for h in range(2):
        S = np.zeros((80, 32))
        for i in range(8):
            S[64 + i, 16 * h + i] = 1.0             # V_i
            for j in range(8):
                S[i * 8 + j, 16 * h + i] = -QP[i, j]
                S[i * 8 + j, 16 * h + 8 + i] = QP[i, j]
        C[f"S_sm{h}"] = S
    # smalls B stationary per group h: [80, 16], content at cols 8h:8h+8
    for h in range(2):
        S = np.zeros((80, 16))
        for j in range(8):
            S[64 + j, 8 * h + j] = 1.0              # V_j
            S[72 + j, 8 * h + j] = mf[j, 0]         # mini*mf
        C[f"S_va{h}"] = S

    # V broadcast over j (from xT): SVB[64+i, i*8+j] = 1
    SVB = np.zeros((80, 64))
    for i in range(8):
        for j in range(8):
            SVB[64 + i, i * 8 + j] = 1.0
    C["SVB"] = SVB

    C["I128"] = np.eye(128)

    # rowsum(g0) into smalls A, 2-group: [128, 32]
    R2 = np.zeros((128, 32))
    for h in range(2):
        for i in range(8):
            for j in range(8):
                R2[h * 64 + i * 8 + j, 16 * h + i] = -1.0
                R2[h * 64 + i * 8 + j, 16 * h + 8 + i] = 1.0
    C["R2"] = R2

    # diag scatter of remain, 2-group: [32, 128]
    DG = np.zeros((32, 128))
    for h in range(2):
        for i in range(8):
            DG[16 * h + i, 64 * h + i * 8 + i] = 1.0
    C["DIAG"] = DG

    # tot broadcast (remain+rsg over j), 2-group: [32, 128]
    TB = np.zeros((32, 128))
    for h in range(2):
        for k in range(16):
            i = k % 8
            for j in range(8):
                TB[16 * h + k, 64 * h + i * 8 + j] = 1.0
    C["TOTB"] = TB

    # fv accumulation from raw (-DepF rowsum + ArrF colsum), 2-group [128, 16]
    CD = np.zeros((128, 16))
    RI = np.zeros((128, 16))
    for h in range(2):
        for i in range(8):
            for j in range(8):
                CD[h * 64 + i * 8 + j, 8 * h + i] += -DepF[i, j]
                CD[h * 64 + i * 8 + j, 8 * h + j] += ArrF[i, j]
                RI[h * 64 + i * 8 + j, 8 * h + i] += -IntF[i, j]
    C["CARD"] = CD
    C["RINT"] = RI

    # no_remain broadcast with PF weight, 2-group [16, 128]
    BP = np.zeros((16, 128))
    for h in range(2):
        for i in range(8):
            for j in range(8):
                BP[8 * h + i, 64 * h + i * 8 + j] = PF[i, j]
    C["B1PF"] = BP

    C["negPF"] = -PF.reshape(64)
    return C


def numpy_model(x, C):
    """Pure-numpy emulation of the device dataflow (algebra validation).
    x: [B, 80] ORIGINAL order. Returns [B,128]: 0:64 action_ij, 64:128 price."""
    x = np.asarray(x, np.float32)[:, OLD_OF_NEW].astype(np.float64)
    Bn = x.shape[0]
    diff = x @ C["Wd"] + C["bias_d"]
    g0 = np.maximum(diff, 0.0)
    t2 = x @ C["S_qqp"]
    smA = x @ C["S_sm0"][:, 0:16] + g0 @ C["R2"][0:64, 0:16]
    sm = np.maximum(smA, 0.0)
    remain, rsg = sm[:, 0:8], sm[:, 8:16]
    gradient = g0 + t2
    gradient[:, 0::9] += remain
    tot_b = np.concatenate([remain, rsg], 1) @ C["TOTB"][0:16, 0:64]
    action = gradient / tot_b
    vb = x @ C["SVB"]
    raw = action * vb
    fg = gradient - raw
    fq = np.maximum(x[:, 0:64] - raw, 0.0)
    smB = x @ C["S_va0"][:, 0:8] + raw @ C["CARD"][0:64, 0:8] \
        + fg @ C["RINT"][0:64, 0:8]
    nr = np.maximum(smB * (1.0 / 7.0), 0.0)
    nrb = nr @ C["B1PF"][0:8, 0:64]
    i1 = fg - fq
    u1 = i1 * C["negPF"] + 1.0
    u2 = u1 - nrb
    price = np.maximum(np.minimum(u2, 1.0), 0.6)
    out = np.empty((Bn, 128), np.float32)
    out[:, 0:64] = action
    out[:, 64:128] = price
    return out


def ref_col_perm():
    """perm such that final[:, c] = mine[:, perm[c]] matches reference layout."""
    perm = np.empty(128, np.int64)
    for i in range(8):
        for j in range(8):
            perm[i * 16 + j] = i * 8 + j
            perm[i * 16 + 8 + j] = 64 + i * 8 + j
    return perm


# device constant blob: pack all bf16 matrices as [128, ncols] column blocks
_BLOB_SPECS = [  # (name, rows, cols)
    ("Wd", 80, 64), ("S_qqp", 80, 64), ("S_sm0", 80, 32), ("S_sm1", 80, 32),
    ("S_va0", 80, 16), ("S_va1", 80, 16), ("SVB", 80, 64), ("I128", 128, 128),
    ("R2", 128, 32), ("DIAG", 32, 128), ("TOTB", 32, 128),
    ("CARD", 128, 16), ("RINT", 128, 16), ("B1PF", 16, 128),
]
_BLOB_OFF = {}
_off = 0
for _n, _r, _c in _BLOB_SPECS:
    _BLOB_OFF[_n] = (_off, _r, _c)
    _off += _c
BLOB_COLS = _off


def pack_blob(C):
    import ml_dtypes

    blob = np.zeros((128, BLOB_COLS), np.float32)
    for n, (o, r, c) in _BLOB_OFF.items():
        blob[0:r, o:o + c] = C[n]
    vec = np.zeros((128, 2), np.float32)
    vec[0:64, 0] = C["bias_d"]
    vec[64:128, 0] = C["bias_d"]
    vec[0:64, 1] = C["negPF"]
    vec[64:128, 1] = C["negPF"]
    return np.ascontiguousarray(blob).astype(ml_dtypes.bfloat16), vec


def _build_nc():
    import concourse.bacc as bacc
    import concourse.tile as tile
    from concourse import mybir

    f32 = mybir.dt.float32
    bf16 = mybir.dt.bfloat16
    RELU = mybir.ActivationFunctionType.Relu
    ALU = mybir.AluOpType

    nc = bacc.Bacc()
    xt_d = nc.declare_dram_parameter("xt", [80, BC], bf16, isOutput=False)
    cst_d = nc.declare_dram_parameter("cst", [128, BLOB_COLS], bf16, isOutput=False)
    cstv_d = nc.declare_dram_parameter("cstv", [128, 2], f32, isOutput=False)
    out_d = nc.declare_dram_parameter("out", [128, BC], f32, isOutput=True)

    with tile.TileContext(nc) as tc:
        with (
            tc.tile_pool(name="const", bufs=1) as cpool,
            tc.tile_pool(name="io", bufs=4) as iopool,
            tc.tile_pool(name="work", bufs=5) as wpool,
            tc.tile_pool(name="ps1", bufs=1, space="PSUM") as pp1,
            tc.tile_pool(name="ps2", bufs=2, space="PSUM") as pp2,
        ):
            cst = cpool.tile([128, BLOB_COLS], bf16)
            cstv = cpool.tile([128, 2], f32)
            nc.sync.dma_start(out=cst[:, :], in_=cst_d[:, :])
            nc.sync.dma_start(out=cstv[:, :], in_=cstv_d[:, :])

            def cs(name):
                o, r, c = _BLOB_OFF[name]
                return cst[0:r, o:o + c]

            biasd_a = cstv[0:128, 0:1]
            negpf_a = cstv[0:128, 1:2]

            for st in range(NST):
                b0c = st * STB
                xt = iopool.tile([80, STB], bf16, tag="xt")
                nc.sync.dma_start(out=xt[:, :], in_=xt_d[:, b0c:b0c + STB])

                fronts = []
                for pr in range(2):
                    diffb = pp1.tile([128, F], f32, tag="diff")
                    gradb = pp2.tile([128, F], f32, tag="grad")
                    g0t = wpool.tile([128, F], bf16, tag="g0")
                    sm_sb = wpool.tile([32, F], bf16, tag="sm_sb")

                    for h in range(2):
                        g = pr * 2 + h
                        xg = xt[:, g * F:(g + 1) * F]
                        po = h * 64
                        nc.tensor.matmul(out=diffb[po:po + 64, :], lhsT=cs("Wd"),
                                         rhs=xg, start=True, stop=True)
                        nc.tensor.matmul(out=gradb[po:po + 64, :], lhsT=cs("S_qqp"),
                                         rhs=xg, start=True, stop=False)
                    # g0 = relu(diff + bias_d) on ACT
                    nc.scalar.activation(out=g0t[:, :], in_=diffb[:, :], func=RELU,
                                         bias=biasd_a, scale=1.0)
                    sma = pp1.tile([128, F], f32, tag="sma")
                    for h in range(2):
                        g = pr * 2 + h
                        nc.tensor.matmul(out=sma[0:32, :], lhsT=cs(f"S_sm{h}"),
                                         rhs=xt[:, g * F:(g + 1) * F],
                                         start=(h == 0), stop=False)
                    nc.tensor.matmul(out=gradb[:, :], lhsT=cs("I128"), rhs=g0t[:, :],
                                     start=False, stop=False)
                    nc.tensor.matmul(out=sma[0:32, :], lhsT=cs("R2"), rhs=g0t[:, :],
                                     start=False, stop=True)
                    # smalls A: {remain, rsg} = relu(sma) on ACT
                    nc.scalar.activation(out=sm_sb[:, :], in_=sma[0:32, :], func=RELU)
                    fronts.append((gradb, sm_sb))

                for pr in range(2):
                    gradb, sm_sb = fronts[pr]
                    totb = pp1.tile([128, F], f32, tag="bcast")
                    actt = wpool.tile([128, F], f32, tag="act")
                    rawt = wpool.tile([128, F], bf16, tag="raw")
                    fgt = wpool.tile([128, F], bf16, tag="fg")
                    fq0 = wpool.tile([128, F], bf16, tag="fq0")
                    fqt = wpool.tile([128, F], bf16, tag="fq")
                    rtb = wpool.tile([128, F], f32, tag="rtb")
                    q2 = wpool.tile([128, F], bf16, tag="q2")
                    nc.tensor.matmul(out=gradb[:, :], lhsT=cs("DIAG"),
                                     rhs=sm_sb[0:32, :], start=False, stop=True)
                    nc.tensor.matmul(out=totb[:, :], lhsT=cs("TOTB"),
                                     rhs=sm_sb[0:32, :], start=True, stop=True)
                    nc.vector.reciprocal_approx_fast(out=rtb[:, :], in_=totb[:, :])
                    vbb = pp1.tile([128, F], f32, tag="vb")
                    for h in range(2):
                        g = pr * 2 + h
                        nc.tensor.matmul(out=vbb[h * 64:h * 64 + 64, :],
                                         lhsT=cs("SVB"),
                                         rhs=xt[:, g * F:(g + 1) * F],
                                         start=True, stop=True)
                    nc.vector.tensor_mul(out=actt[:, :], in0=gradb[:, :], in1=rtb[:, :])
                    nc.vector.tensor_mul(out=rawt[:, :], in0=actt[:, :], in1=vbb[:, :])
                    nc.vector.tensor_sub(out=fgt[:, :], in0=gradb[:, :], in1=rawt[:, :])
                    for h in range(2):
                        g = pr * 2 + h
                        nc.gpsimd.tensor_copy(out=q2[h * 64:h * 64 + 64, :],
                                              in_=xt[0:64, g * F:(g + 1) * F])
                    # fq = relu(Q - raw): sub on DVE, relu on ACT
                    nc.vector.tensor_sub(out=fq0[:, :], in0=q2[:, :], in1=rawt[:, :])
                    nc.scalar.activation(out=fqt[:, :], in_=fq0[:, :], func=RELU)
                    smb = pp1.tile([128, F], f32, tag="smb")
                    smb_sb = wpool.tile([16, F], bf16, tag="smb_sb")
                    for h in range(2):
                        g = pr * 2 + h
                        nc.tensor.matmul(out=smb[0:16, :], lhsT=cs(f"S_va{h}"),
                                         rhs=xt[:, g * F:(g + 1) * F],
                                         start=(h == 0), stop=False)
                    nc.tensor.matmul(out=smb[0:16, :], lhsT=cs("CARD"),
                                     rhs=rawt[:, :], start=False, stop=False)
                    nc.tensor.matmul(out=smb[0:16, :], lhsT=cs("RINT"),
                                     rhs=fgt[:, :], start=False, stop=True)
                    # smalls B: no_remain = relu(fv_minus_int / 7) on ACT
                    nc.scalar.activation(out=smb_sb[:, :], in_=smb[0:16, :],
                                         func=RELU, scale=1.0 / 7.0)

                    nrb = pp1.tile([128, F], f32, tag="bcast2")
                    i1t = wpool.tile([128, F], bf16, tag="i1")
                    prt = wpool.tile([128, F], f32, tag="prc")
                    nc.tensor.matmul(out=nrb[:, :], lhsT=cs("B1PF"),
                                     rhs=smb_sb[0:16, :], start=True, stop=True)
                    nc.vector.tensor_sub(out=i1t[:, :], in0=fgt[:, :], in1=fqt[:, :])
                    nc.vector.tensor_scalar(out=i1t[:, :], in0=i1t[:, :],
                                            scalar1=negpf_a, scalar2=1.0,
                                            op0=ALU.mult, op1=ALU.add)
                    nc.vector.tensor_sub(out=prt[:, :], in0=i1t[:, :], in1=nrb[:, :])
                    nc.vector.tensor_scalar(out=prt[:, :], in0=prt[:, :],
                                            scalar1=1.0, scalar2=0.6,
                                            op0=ALU.min, op1=ALU.max)
                    for h in range(2):
                        g = pr * 2 + h
                        po = h * 64
                        bg = b0c + g * F
                        nc.sync.dma_start(out=out_d[0:64, bg:bg + F],
                                          in_=actt[po:po + 64, :])
                        nc.sync.dma_start(out=out_d[64:128, bg:bg + F],
                                          in_=prt[po:po + 64, :])
    nc.finalize()
    return nc


_CACHE = {}


def kernel(**inputs):
    global LAST_RESULT
    x = np.ascontiguousarray(np.asarray(inputs["x"], np.float32))
    C = build_consts(
        inputs["W0"], inputs["b0"], inputs["W1"], inputs["b1"],
        inputs["distribute_param"], inputs["queue_param"],
        inputs["departure_factor"], inputs["arrival_factor"],
        inputs["mini_factor"], inputs["intention_factor"],
        inputs["price_factor"],
    )
    blob, vec = pack_blob(C)

    if "nc" not in _CACHE:
        _CACHE["nc"] = _build_nc()
    nc = _CACHE["nc"]

    from concourse.bass_utils import run_bass_kernel_spmd

    import ml_dtypes

    bf = ml_dtypes.bfloat16
    in_maps = []
    for c in range(NCORES):
        shard = np.ascontiguousarray(
            x[c * BC:(c + 1) * BC][:, OLD_OF_NEW].T).astype(bf)
        in_maps.append({"xt": shard, "cst": blob, "cstv": vec})

    trace = bool(int(os.environ.get("KBENCH_TRACE", "0")))
    if trace:
        try:
            import ntff_shim

            ntff_shim.install()
        except Exception as e:  # profiling is best-effort
            print(f"ntff shim install failed: {e}")
    res = run_bass_kernel_spmd(nc, in_maps, core_ids=list(range(NCORES)),
                               trace=trace)
    LAST_RESULT = res

    perm = ref_col_perm()
    out = np.empty((B, 128), np.float32)
    for c in range(NCORES):
        mine = res.results[c]["out"].T  # [BC, 128] in my column order
        out[c * BC:(c + 1) * BC, :] = mine[:, perm]
    return out


# revision 26
# speedup vs baseline: 1.2114x; 1.1645x over previous
"""Trainium2 Bass kernel for nn_ActionNetwork (dense_mlp, 8-core data parallel).

Layout strategy: feature-on-partition, batch-on-free-dim.
  - Host transposes x [B,80] -> xT [80,B] (feature rows reordered to
    [queue(64), vehicle(8), mini(8)]), shards batch across 8 cores, bf16.
  - The potential network (2 tiny linears) is linear in x, so it is folded
    into one [80,64] stationary producing diff*DP directly from xT.
  - All broadcasts (over j), reductions (row/col sums) and the diag scatter
    are tiny PE matmuls with precomputed 0/1-weighted bf16 stationaries.
  - Elementwise chain is split DVE/ACT/Pool over [128, 512] tiles packing
    2 batch-groups (64 (i,j)-pairs each).
  - Narrow 8-wide per-node quantities pack 2 groups contiguously (rows 0:32
    resp 0:16) in one PSUM bank; group-dependent zero-padded stationaries
    let each per-group xT matmul land on its own row range while later
    2-group matmuls accumulate the whole range at once.
Output written as outT [128, Bc] (rows 0:64 action_ij, 64:128 price_ij),
host transposes back and permutes columns into the reference interleaved
[action_i | price_i] * 8 ordering.
"""

import os
import sys

import numpy as np

sys.path.insert(0, "/opt/trn_rl_repo")

N = 8
B = 262144
NCORES = 8
BC = B // NCORES  # 32768 per core
F = 512           # batch elements per group (one PSUM bank of f32)
STB = F * 4       # batch per supertile = 2048 (2 pairs x 2 groups)
NST = BC // STB   # 16 supertiles per core

LAST_RESULT = None  # test harness reads exec_time_ns from here

OLD_OF_NEW = np.concatenate([np.arange(16, 80), np.arange(0, 8),
                             np.arange(8, 16)])  # xT feature row order


def build_consts(W0, b0, W1, b1, DP, QP, DepF, ArrF, mf, IntF, PF):
    """Build all constant matrices (float64, logical layout).

    xT feature order is [queue(64), vehicle(8), mini(8)]:
      rows 0:64 queue_ij, 64:72 vehicle_i, 72:80 mini_i.
    """
    W0 = np.asarray(W0, np.float64)
    W1 = np.asarray(W1, np.float64)
    b0 = np.asarray(b0, np.float64)
    b1 = np.asarray(b1, np.float64)
    DP = np.asarray(DP, np.float64)
    QP = np.asarray(QP, np.float64)
    DepF = np.asarray(DepF, np.float64)
    ArrF = np.asarray(ArrF, np.float64)
    mf = np.asarray(mf, np.float64)
    IntF = np.asarray(IntF, np.float64)
    PF = np.asarray(PF, np.float64)

    W0full = np.zeros((80, 8))  # ORIGINAL x feature order first
    for i in range(8):
        W0full[i, i] = W0[i, 0]
        W0full[8 + i, i] = W0[i, 1]
        for j in range(8):
            W0full[16 + i * 8 + j, i] += W0[i, 2 + j]        # queue[i, j]
            W0full[16 + j * 8 + i, i] += W0[i, 2 + 8 + j]    # queue[j, i]
    Wp = W0full @ W1.T            # [80, 8]
    bp = b0 @ W1.T + b1           # [8]

    C = {}
    # Wd[f, ij] = (Wp[f,i] - Wp[f,j]) * DP[i,j], in new row order
    Wd = (Wp[:, :, None] - Wp[:, None, :]).reshape(80, 64) * DP.reshape(64)[None, :]
    C["Wd"] = Wd[OLD_OF_NEW]
    C["bias_d"] = ((bp[:, None] - bp[None, :]) * DP).reshape(64)

    S_qqp = np.zeros((80, 64))   # new row order directly: queue at rows 0:64
    for ij in range(64):
        S_qqp[ij, ij] = QP.reshape(64)[ij]
    C["S_qqp"] = S_qqp

    # smalls A stationary per group h: [80, 32], content at cols 16h:16h+16
    for h in range(2):
        S = np.zeros((80, 32))
        for i in range(8):
            S[64 + i, 16 * h + i] = 1.0             # V_i
            for j in range(8):
                S[i * 8 + j, 16 * h + i] = -QP[i, j]
                S[i * 8 + j, 16 * h + 8 + i] = QP[i, j]
        C[f"S_sm{h}"] = S
    # smalls B stationary per group h: [80, 16], content at cols 8h:8h+8
    for h in range(2):
        S = np.zeros((80, 16))
        for j in range(8):
            S[64 + j, 8 * h + j] = 1.0              # V_j
            S[72 + j, 8 * h + j] = mf[j, 0]         # mini*mf
        C[f"S_va{h}"] = S

    # V broadcast over j (from xT): SVB[64+i, i*8+j] = 1
    SVB = np.zeros((80, 64))
    for i in range(8):
        for j in range(8):
            SVB[64 + i, i * 8 + j] = 1.0
    C["SVB"] = SVB

    C["I128"] = np.eye(128)

    # rowsum(g0) into smalls A, 2-group: [128, 32]
    R2 = np.zeros((128, 32))
    for h in range(2):
        for i in range(8):
            for j in range(8):
                R2[h * 64 + i * 8 + j, 16 * h + i] = -1.0
                R2[h * 64 + i * 8 + j, 16 * h + 8 + i] = 1.0
    C["R2"] = R2

    # diag scatter of remain, 2-group: [32, 128]
    DG = np.zeros((32, 128))
    for h in range(2):
        for i in range(8):
            DG[16 * h + i, 64 * h + i * 8 + i] = 1.0
    C["DIAG"] = DG

    # tot broadcast (remain+rsg over j), 2-group: [32, 128]
    TB = np.zeros((32, 128))
    for h in range(2):
        for k in range(16):
            i = k % 8
            for j in range(8):
                TB[16 * h + k, 64 * h + i * 8 + j] = 1.0
    C["TOTB"] = TB

    # fv accumulation from raw (-DepF rowsum + ArrF colsum), 2-group [128, 16]
    CD = np.zeros((128, 16))
    RI = np.zeros((128, 16))
    for h in range(2):
        for i in range(8):
            for j in range(8):
                CD[h * 64 + i * 8 + j, 8 * h + i] += -DepF[i, j]
                CD[h * 64 + i * 8 + j, 8 * h + j] += ArrF[i, j]
                RI[h * 64 + i * 8 + j, 8 * h + i] += -IntF[i, j]
    C["CARD"] = CD
    C["RINT"] = RI

    # no_remain broadcast with PF weight, 2-group [16, 128]
    BP = np.zeros((16, 128))
    for h in range(2):
        for i in range(8):
            for j in range(8):
                BP[8 * h + i, 64 * h + i * 8 + j] = PF[i, j]
    C["B1PF"] = BP

    C["negPF"] = -PF.reshape(64)
    return C


def numpy_model(x, C):
    """Pure-numpy emulation of the device dataflow (algebra validation).
    x: [B, 80] ORIGINAL order. Returns [B,128]: 0:64 action_ij, 64:128 price."""
    x = np.asarray(x, np.float32)[:, OLD_OF_NEW].astype(np.float64)
    Bn = x.shape[0]
    diff = x @ C["Wd"] + C["bias_d"]
    g0 = np.maximum(diff, 0.0)
    t2 = x @ C["S_qqp"]
    smA = x @ C["S_sm0"][:, 0:16] + g0 @ C["R2"][0:64, 0:16]
    sm = np.maximum(smA, 0.0)
    remain, rsg = sm[:, 0:8], sm[:, 8:16]
    gradient = g0 + t2
    gradient[:, 0::9] += remain
    tot_b = np.concatenate([remain, rsg], 1) @ C["TOTB"][0:16, 0:64]
    action = gradient / tot_b
    vb = x @ C["SVB"]
    raw = action * vb
    fg = gradient - raw
    fq = np.maximum(x[:, 0:64] - raw, 0.0)
    smB = x @ C["S_va0"][:, 0:8] + raw @ C["CARD"][0:64, 0:8] \
        + fg @ C["RINT"][0:64, 0:8]
    nr = np.maximum(smB * (1.0 / 7.0), 0.0)
    nrb = nr @ C["B1PF"][0:8, 0:64]
    i1 = fg - fq
    u1 = i1 * C["negPF"] + 1.0
    u2 = u1 - nrb
    price = np.maximum(np.minimum(u2, 1.0), 0.6)
    out = np.empty((Bn, 128), np.float32)
    out[:, 0:64] = action
    out[:, 64:128] = price
    return out


def ref_col_perm():
    """perm such that final[:, c] = mine[:, perm[c]] matches reference layout."""
    perm = np.empty(128, np.int64)
    for i in range(8):
        for j in range(8):
            perm[i * 16 + j] = i * 8 + j
            perm[i * 16 + 8 + j] = 64 + i * 8 + j
    return perm


# device constant blob: pack all bf16 matrices as [128, ncols] column blocks
_BLOB_SPECS = [  # (name, rows, cols)
    ("Wd", 80, 64), ("S_qqp", 80, 64), ("S_sm0", 80, 32), ("S_sm1", 80, 32),
    ("S_va0", 80, 16), ("S_va1", 80, 16), ("SVB", 80, 64), ("I128", 128, 128),
    ("R2", 128, 32), ("DIAG", 32, 128), ("TOTB", 32, 128),
    ("CARD", 128, 16), ("RINT", 128, 16), ("B1PF", 16, 128),
]
_BLOB_OFF = {}
_off = 0
for _n, _r, _c in _BLOB_SPECS:
    _BLOB_OFF[_n] = (_off, _r, _c)
    _off += _c
BLOB_COLS = _off


def pack_blob(C):
    import ml_dtypes

    blob = np.zeros((128, BLOB_COLS), np.float32)
    for n, (o, r, c) in _BLOB_OFF.items():
        blob[0:r, o:o + c] = C[n]
    vec = np.zeros((128, 2), np.float32)
    vec[0:64, 0] = C["bias_d"]
    vec[64:128, 0] = C["bias_d"]
    vec[0:64, 1] = C["negPF"]
    vec[64:128, 1] = C["negPF"]
    return np.ascontiguousarray(blob).astype(ml_dtypes.bfloat16), vec


def _build_nc():
    import concourse.bacc as bacc
    import concourse.tile as tile
    from concourse import mybir

    f32 = mybir.dt.float32
    bf16 = mybir.dt.bfloat16
    RELU = mybir.ActivationFunctionType.Relu
    ALU = mybir.AluOpType

    nc = bacc.Bacc()
    xt_d = nc.declare_dram_parameter("xt", [80, BC], bf16, isOutput=False)
    cst_d = nc.declare_dram_parameter("cst", [128, BLOB_COLS], bf16, isOutput=False)
    cstv_d = nc.declare_dram_parameter("cstv", [128, 2], f32, isOutput=False)
    out_d = nc.declare_dram_parameter("out", [128, BC], f32, isOutput=True)

    with tile.TileContext(nc) as tc:
        with (
            tc.tile_pool(name="const", bufs=1) as cpool,
            tc.tile_pool(name="io", bufs=4) as iopool,
            tc.tile_pool(name="work", bufs=5) as wpool,
            tc.tile_pool(name="ps1", bufs=1, space="PSUM") as pp1,
            tc.tile_pool(name="ps2", bufs=3, space="PSUM") as pp2,
        ):
            cst = cpool.tile([128, BLOB_COLS], bf16)
            cstv = cpool.tile([128, 2], f32)
            nc.sync.dma_start(out=cst[:, :], in_=cst_d[:, :])
            nc.sync.dma_start(out=cstv[:, :], in_=cstv_d[:, :])

            def cs(name):
                o, r, c = _BLOB_OFF[name]
                return cst[0:r, o:o + c]

            biasd_a = cstv[0:128, 0:1]
            negpf_a = cstv[0:128, 1:2]

            for st in range(NST):
                b0c = st * STB
                xt = iopool.tile([80, STB], bf16, tag="xt")
                nc.sync.dma_start(out=xt[:, :], in_=xt_d[:, b0c:b0c + STB])

                fronts = []
                for pr in range(2):
                    diffb = pp2.tile([128, F], f32, tag="dg")
                    gradb = pp2.tile([128, F], f32, tag="dg")
                    g0t = wpool.tile([128, F], bf16, tag="g0")
                    sm_sb = wpool.tile([32, F], bf16, tag="sm_sb")

                    for h in range(2):
                        g = pr * 2 + h
                        xg = xt[:, g * F:(g + 1) * F]
                        po = h * 64
                        nc.tensor.matmul(out=diffb[po:po + 64, :], lhsT=cs("Wd"),
                                         rhs=xg, start=True, stop=True)
                        nc.tensor.matmul(out=gradb[po:po + 64, :], lhsT=cs("S_qqp"),
                                         rhs=xg, start=True, stop=False)
                    # g0 = relu(diff + bias_d) on ACT
                    nc.scalar.activation(out=g0t[:, :], in_=diffb[:, :], func=RELU,
                                         bias=biasd_a, scale=1.0)
                    sma = pp1.tile([128, F], f32, tag="sma")
                    for h in range(2):
                        g = pr * 2 + h
                        nc.tensor.matmul(out=sma[0:32, :], lhsT=cs(f"S_sm{h}"),
                                         rhs=xt[:, g * F:(g + 1) * F],
                                         start=(h == 0), stop=False)
                    nc.tensor.matmul(out=gradb[:, :], lhsT=cs("I128"), rhs=g0t[:, :],
                                     start=False, stop=False)
                    nc.tensor.matmul(out=sma[0:32, :], lhsT=cs("R2"), rhs=g0t[:, :],
                                     start=False, stop=True)
                    # smalls A: {remain, rsg} = relu(sma) on ACT
                    nc.scalar.activation(out=sm_sb[:, :], in_=sma[0:32, :], func=RELU)
                    fronts.append((gradb, sm_sb))

                for pr in range(2):
                    gradb, sm_sb = fronts[pr]
                    totb = pp1.tile([128, F], f32, tag="bcast")
                    actt = wpool.tile([128, F], f32, tag="act")
                    rawt = wpool.tile([128, F], bf16, tag="raw")
                    fgt = wpool.tile([128, F], bf16, tag="fg")
                    fq0 = wpool.tile([128, F], bf16, tag="fq0")
                    fqt = wpool.tile([128, F], bf16, tag="fq")
                    rtb = wpool.tile([128, F], f32, tag="rtb")
                    q2 = wpool.tile([128, F], bf16, tag="q2")
                    nc.tensor.matmul(out=gradb[:, :], lhsT=cs("DIAG"),
                                     rhs=sm_sb[0:32, :], start=False, stop=True)
                    nc.tensor.matmul(out=totb[:, :], lhsT=cs("TOTB"),
                                     rhs=sm_sb[0:32, :], start=True, stop=True)
                    nc.vector.reciprocal_approx_fast(out=rtb[:, :], in_=totb[:, :])
                    vbb = pp1.tile([128, F], f32, tag="vb")
                    for h in range(2):
                        g = pr * 2 + h
                        nc.tensor.matmul(out=vbb[h * 64:h * 64 + 64, :],
                                         lhsT=cs("SVB"),
                                         rhs=xt[:, g * F:(g + 1) * F],
                                         start=True, stop=True)
                    nc.vector.tensor_mul(out=actt[:, :], in0=gradb[:, :], in1=rtb[:, :])
                    nc.vector.tensor_mul(out=rawt[:, :], in0=actt[:, :], in1=vbb[:, :])
                    nc.vector.tensor_sub(out=fgt[:, :], in0=gradb[:, :], in1=rawt[:, :])
                    for h in range(2):
                        g = pr * 2 + h
                        nc.gpsimd.tensor_copy(out=q2[h * 64:h * 64 + 64, :],
                                              in_=xt[0:64, g * F:(g + 1) * F])
                    # fq = relu(Q - raw): sub on DVE, relu on ACT
                    nc.vector.tensor_sub(out=fq0[:, :], in0=q2[:, :], in1=rawt[:, :])
                    nc.scalar.activation(out=fqt[:, :], in_=fq0[:, :], func=RELU)
                    smb = pp1.tile([128, F], f32, tag="smb")
                    smb_sb = wpool.tile([16, F], bf16, tag="smb_sb")
                    for h in range(2):
                        g = pr * 2 + h
                        nc.tensor.matmul(out=smb[0:16, :], lhsT=cs(f"S_va{h}"),
                                         rhs=xt[:, g * F:(g + 1) * F],
                                         start=(h == 0), stop=False)
                    nc.tensor.matmul(out=smb[0:16, :], lhsT=cs("CARD"),
                                     rhs=rawt[:, :], start=False, stop=False)
                    nc.tensor.matmul(out=smb[0:16, :], lhsT=cs("RINT"),
                                     rhs=fgt[:, :], start=False, stop=True)
                    # smalls B: no_remain = relu(fv_minus_int / 7) on ACT
                    nc.scalar.activation(out=smb_sb[:, :], in_=smb[0:16, :],
                                         func=RELU, scale=1.0 / 7.0)

                    nrb = pp1.tile([128, F], f32, tag="bcast2")
                    i1t = wpool.tile([128, F], bf16, tag="i1")
                    prt = wpool.tile([128, F], f32, tag="prc")
                    nc.tensor.matmul(out=nrb[:, :], lhsT=cs("B1PF"),
                                     rhs=smb_sb[0:16, :], start=True, stop=True)
                    nc.vector.tensor_sub(out=i1t[:, :], in0=fgt[:, :], in1=fqt[:, :])
                    nc.vector.tensor_scalar(out=i1t[:, :], in0=i1t[:, :],
                                            scalar1=negpf_a, scalar2=1.0,
                                            op0=ALU.mult, op1=ALU.add)
                    nc.vector.tensor_sub(out=prt[:, :], in0=i1t[:, :], in1=nrb[:, :])
                    nc.vector.tensor_scalar(out=prt[:, :], in0=prt[:, :],
                                            scalar1=1.0, scalar2=0.6,
                                            op0=ALU.min, op1=ALU.max)
                    for h in range(2):
                        g = pr * 2 + h
                        po = h * 64
                        bg = b0c + g * F
                        nc.sync.dma_start(out=out_d[0:64, bg:bg + F],
                                          in_=actt[po:po + 64, :])
                        nc.sync.dma_start(out=out_d[64:128, bg:bg + F],
                                          in_=prt[po:po + 64, :])
    nc.finalize()
    return nc


_CACHE = {}


def kernel(**inputs):
    global LAST_RESULT
    x = np.ascontiguousarray(np.asarray(inputs["x"], np.float32))
    C = build_consts(
        inputs["W0"], inputs["b0"], inputs["W1"], inputs["b1"],
        inputs["distribute_param"], inputs["queue_param"],
        inputs["departure_factor"], inputs["arrival_factor"],
        inputs["mini_factor"], inputs["intention_factor"],
        inputs["price_factor"],
    )
    blob, vec = pack_blob(C)

    if "nc" not in _CACHE:
        _CACHE["nc"] = _build_nc()
    nc = _CACHE["nc"]

    from concourse.bass_utils import run_bass_kernel_spmd

    import ml_dtypes

    bf = ml_dtypes.bfloat16
    in_maps = []
    for c in range(NCORES):
        shard = np.ascontiguousarray(
            x[c * BC:(c + 1) * BC][:, OLD_OF_NEW].T).astype(bf)
        in_maps.append({"xt": shard, "cst": blob, "cstv": vec})

    trace = bool(int(os.environ.get("KBENCH_TRACE", "0")))
    if trace:
        try:
            import ntff_shim

            ntff_shim.install()
        except Exception as e:  # profiling is best-effort
            print(f"ntff shim install failed: {e}")
    res = run_bass_kernel_spmd(nc, in_maps, core_ids=list(range(NCORES)),
                               trace=trace)
    LAST_RESULT = res

    perm = ref_col_perm()
    out = np.empty((B, 128), np.float32)
    for c in range(NCORES):
        mine = res.results[c]["out"].T  # [BC, 128] in my column order
        out[c * BC:(c + 1) * BC, :] = mine[:, perm]
    return out
